# revision 15
# baseline (speedup 1.0000x reference)
"""Causal multi-head attention (B=4, S=2048, D=1024, H=16, Dh=64) on 8 TRN2
NeuronCores.

Sharding: core c -> batch b = c//2, head group hg = c%2 (8 heads each).
Each core computes the partial output (sum over its 8 heads) TRANSPOSED:
OT_partial [D=1024, S=2048] in fp32.  Host sums the two partials per batch
and transposes back.

v2: software-pipelined issue order.  The attention inner loop previously
issued scores -> exp -> mask -> PV per k-block in program order, so the
in-order PE queue stalled behind ACT's exp every k-block.  Now PV lags one
k-block behind scores, and projection matmuls (Q/K/V/O) are issued as
single-MM "filler" units between the scores and PV of each k-block, so PE
always has independent work while ACT computes exp.  Fillers pending at a
(pair, q-tile) boundary are flushed there, which keeps the producer-before-
consumer order that the in-order engine queues need to stay deadlock-free.

Per-core kernel (bf16 matmuls, fp32 PSUM accumulation):
  XT  = residual[b].T (bf16)          [1024(m), 2048(s)]  (host-pretransposed)
  WQT/WKT/WVT (bf16) [1024(m), 512(h*64+d)]              (host-pretransposed)
  WOS = W_O[heads].reshape (bf16)     [512(h*64+d), 1024(m)]
  QT/KT (pair-packed) [128=2x64(hd), 2048(s)] x 4 pairs; score matmuls are
      row-tiled (heads at array rows 0-63 / 64-127)
  V    [128(k within chunk), 8*65] x 16 chunks (ones col per head: the PV
      matmul's 65th output row accumulates sum(exp) for free)
  scoresT [k,q] tiles -> one ACT exp per k-block covering both heads
      (scale=1/8; no max-subtraction -- scores are bounded ~+-2.5 for this
      input distribution)
  causal: staircase-restricted matmul widths + affine_select on the
      diagonal 128-blocks only
  PV: psum[65, 512] accumulated over k blocks; row 64 = sumexp
  normalize (partition_broadcast only works from physical partition 0 on
      HW): stage psum->SBUF, DVE reciprocal row64->row0 of a 1-row tile,
      gpsimd partition_broadcast from p0, DVE multiply
  O-proj: lhsT = WOS chunks, rhs = AOT pair tiles -> OT [1024, 2048] fp32
"""

from collections import deque
from contextlib import ExitStack

import ml_dtypes
import numpy as np

import concourse.bacc as bacc
import concourse.mybir as mybir
import concourse.tile as tile
from concourse.bass_utils import run_bass_kernel_spmd

# ---------------------------------------------------------------- constants
B, S, D = 4, 2048, 1024
H, Dh = 16, 64
NCORES = 8
HPC = H // 2          # heads per core = 8
HD = HPC * Dh         # 512
NPAIR = HPC // 2      # 4 head pairs per core
MC = D // 128         # 8 m-chunks
QT_W = 512            # q tile width
NQT = S // QT_W       # 4
SC = S // 128         # 16 s-chunks (k blocks)
VROW = Dh + 1         # 65: per-head V columns incl. ones col
F32 = mybir.dt.float32
BF16 = mybir.dt.bfloat16
NPBF16 = ml_dtypes.bfloat16

_CACHED = {}


def build_kernel(repeat=1, hw_loop=False):
    nc = bacc.Bacc("TRN2", target_bir_lowering=False, debug=False,
                   num_devices=NCORES)

    xt_d = nc.dram_tensor("xt", [D, S], BF16, kind="ExternalInput").ap()
    wqt_d = nc.dram_tensor("wqt", [D, HD], BF16, kind="ExternalInput").ap()
    wkt_d = nc.dram_tensor("wkt", [D, HD], BF16, kind="ExternalInput").ap()
    wvt_d = nc.dram_tensor("wvt", [D, HD], BF16, kind="ExternalInput").ap()
    wos_d = nc.dram_tensor("wos", [HD, D], BF16, kind="ExternalInput").ap()
    ot_d = nc.dram_tensor("ot", [D, S], F32, kind="ExternalOutput").ap()

    with tile.TileContext(nc) as tc, ExitStack() as ctx:
        if hw_loop and repeat > 1:
            ctx.enter_context(tc.For_i(0, repeat, 1, name="rep"))
            repeat = 1
        # ---------------- persistent SBUF tensors -------------------------
        w_pool = ctx.enter_context(tc.tile_pool(name="w", bufs=1))
        qk_pool = ctx.enter_context(tc.tile_pool(name="qk", bufs=1))
        v_pool = ctx.enter_context(tc.tile_pool(name="v", bufs=1))
        aot_pool = ctx.enter_context(tc.tile_pool(name="aot", bufs=1))
        xt_pool = ctx.enter_context(tc.tile_pool(name="xt", bufs=1))
        pt_pool = ctx.enter_context(tc.tile_pool(name="pt", bufs=6))
        stg_pool = ctx.enter_context(tc.tile_pool(name="stg", bufs=4))
        rc_pool = ctx.enter_context(tc.tile_pool(name="rc", bufs=2))
        bc_pool = ctx.enter_context(tc.tile_pool(name="bc", bufs=2))
        psum = ctx.enter_context(tc.tile_pool(name="ps", bufs=1,
                                              space="PSUM"))

        for _rep in range(repeat):
            wq_t = w_pool.tile([128, MC * HD], BF16, tag="wqt")
            wk_t = w_pool.tile([128, MC * HD], BF16, tag="wkt")
            wv_t = w_pool.tile([128, MC * HD], BF16, tag="wvt")
            wo_t = w_pool.tile([128, NPAIR * D], BF16, tag="wot")
            xt_t = xt_pool.tile([128, MC * S], BF16)
            for mc in range(MC):
                nc.sync.dma_start(wq_t[:, mc * HD:(mc + 1) * HD],
                                  wqt_d[mc * 128:(mc + 1) * 128, :])
                nc.sync.dma_start(wk_t[:, mc * HD:(mc + 1) * HD],
                                  wkt_d[mc * 128:(mc + 1) * 128, :])
                nc.sync.dma_start(wv_t[:, mc * HD:(mc + 1) * HD],
                                  wvt_d[mc * 128:(mc + 1) * 128, :])
                nc.sync.dma_start(xt_t[:, mc * S:(mc + 1) * S],
                                  xt_d[mc * 128:(mc + 1) * 128, :])
            for c in range(NPAIR):
                nc.sync.dma_start(wo_t[:, c * D:(c + 1) * D],
                                  wos_d[c * 128:(c + 1) * 128, :])

            # QT pair-packed: [128 (2 heads x 64), S] per pair.  KT is split
            # per head with the OTHER head's 64 rows zeroed (ktz), so the
            # score matmuls contract over a full 128 partitions: 128-row
            # tiling mode is ~100ns/MM faster than 64-row mode on HW and
            # avoids PE tiling-mode switches against the PV/filler matmuls.
            qt_t = [qk_pool.tile([128, S], BF16, tag=f"qt{p}", name=f"qt{p}")
                    for p in range(NPAIR)]
            ktz = [[qk_pool.tile([128, S], BF16, tag=f"ktz{p}_{e}",
                                 name=f"ktz{p}_{e}") for e in range(2)]
                   for p in range(NPAIR)]
            for p in range(NPAIR):
                nc.vector.memset(ktz[p][0][64:128, :], 0.0)
                nc.vector.memset(ktz[p][1][0:64, :], 0.0)
            # V: per s-chunk [128, HPC*VROW]: 8 heads x (64 data + 1 ones)
            v_ts = [v_pool.tile([128, HPC * VROW], BF16, tag=f"v{sc}",
                                name=f"v{sc}") for sc in range(SC)]
            # AOT pair-packed: [128, S] per pair
            aot_t = [aot_pool.tile([128, S], BF16, tag=f"aot{p}",
                                   name=f"aot{p}") for p in range(NPAIR)]

            # -------- filler machinery: single-MM units issued into the
            # gaps of the attention stream ---------------------------------
            fillers = deque()

            def fill(n):
                for _ in range(min(n, len(fillers))):
                    fillers.popleft()()

            def flush():
                while fillers:
                    fillers.popleft()()

            # ---------------- Q/K projection units for one (pair, s-tile)
            def push_qk_units(p, st):
                cell = {}

                def mk_mm(which, mc):
                    def f():
                        if mc == 0:
                            cell[which] = psum.tile(
                                [128, QT_W], F32, tag="qk", bufs=2,
                                name=f"ps_{which}")
                        w = wq_t if which == "q" else wk_t
                        nc.tensor.matmul(
                            cell[which][:],
                            w[:, mc * HD + p * 128: mc * HD + (p + 1) * 128],
                            xt_t[:, mc * S + st * QT_W:
                                 mc * S + (st + 1) * QT_W],
                            start=(mc == 0), stop=(mc == MC - 1))
                    return f

                def mk_copy_q():
                    def f():
                        nc.vector.tensor_copy(
                            qt_t[p][:, st * QT_W:(st + 1) * QT_W],
                            cell["q"][:])
                    return f

                def mk_copy_k(e):
                    def f():
                        hb = e * 64
                        nc.vector.tensor_copy(
                            ktz[p][e][hb:hb + 64, st * QT_W:(st + 1) * QT_W],
                            cell["k"][hb:hb + 64, :])
                    return f

                for mc in range(MC):
                    fillers.append(mk_mm("q", mc))
                for mc in range(MC):
                    fillers.append(mk_mm("k", mc))
                fillers.append(mk_copy_q())
                fillers.append(mk_copy_k(0))
                fillers.append(mk_copy_k(1))

            def qk_block(p, st):
                push_qk_units(p, st)
                flush()

            # ---------------- V projection units (activations stationary)
            def push_v_units(sc):
                cell = {}

                def mk_mm(mc):
                    def f():
                        if mc == 0:
                            cell["v"] = psum.tile(
                                [128, HD], F32, tag="qk", bufs=2, name="ps_v")
                        nc.tensor.matmul(
                            cell["v"][:],
                            xt_t[:, mc * S + sc * 128: mc * S + (sc + 1) * 128],
                            wv_t[:, mc * HD:(mc + 1) * HD],
                            start=(mc == 0), stop=(mc == MC - 1))
                    return f

                def cp():
                    vg = v_ts[sc][:].rearrange("p (h e) -> p h e", h=HPC)
                    nc.vector.tensor_copy(
                        vg[:, :, 0:Dh],
                        cell["v"][:].rearrange("p (h d) -> p h d", h=HPC))
                    nc.vector.memset(vg[:, :, Dh:VROW], 1.0)

                for mc in range(MC):
                    fillers.append(mk_mm(mc))
                fillers.append(cp)

            def v_block(sc):
                push_v_units(sc)
                flush()

            # ---------------- O-projection units (one q-tile, all m) ------
            def push_o_units(ot):
                for mc in range(MC):
                    cell = {}

                    def mk_mm(c, mc=mc, cell=cell):
                        def f():
                            if c == 0:
                                cell["o"] = psum.tile(
                                    [128, QT_W], F32, tag="qk", bufs=2,
                                    name="ps_o")
                            nc.tensor.matmul(
                                cell["o"][:],
                                wo_t[:, c * D + mc * 128:
                                     c * D + (mc + 1) * 128],
                                aot_t[c][:, ot * QT_W:(ot + 1) * QT_W],
                                start=(c == 0), stop=(c == NPAIR - 1))
                        return f

                    def cp(mc=mc, cell=cell):
                        ot_sb = pt_pool.tile([128, QT_W], F32, tag="ott",
                                             bufs=4, name="ot_sb")
                        nc.vector.tensor_copy(ot_sb[:], cell["o"][:])
                        nc.sync.dma_start(
                            ot_d[mc * 128:(mc + 1) * 128,
                                 ot * QT_W:(ot + 1) * QT_W], ot_sb[:])

                    for c in range(NPAIR):
                        fillers.append(mk_mm(c))
                    fillers.append(cp)

            # ---------------- attention for (pair, q-tile) ---------------
            def attention(p, qt):
                nkb = 4 * qt + 4
                ps_pv = [psum.tile([VROW, QT_W], F32, tag=f"pv{e}", bufs=1,
                                   name=f"ps_pv{e}") for e in range(2)]
                pvq = deque()
                for kb in range(nkb):
                    r = kb - 4 * qt
                    cs = max(0, r * 128)  # first valid q col in tile
                    # both heads' scoresT into one 2-bank psum tile
                    ps_s = psum.tile([128, 2 * QT_W], F32, tag="s", bufs=2,
                                     name="ps_s")
                    pt = pt_pool.tile([128, 2 * QT_W], BF16, tag="pt",
                                      name="pt")
                    for e in range(2):
                        nc.tensor.matmul(
                            ps_s[:, e * QT_W + cs:(e + 1) * QT_W],
                            ktz[p][e][:, kb * 128:(kb + 1) * 128],
                            qt_t[p][:, qt * QT_W + cs:(qt + 1) * QT_W],
                            start=True, stop=True)
                    # one exp(scores/8) PSUM -> SBUF for both heads
                    nc.scalar.activation(
                        pt.rearrange("p (e w) -> p e w", e=2)[:, :, cs:QT_W],
                        ps_s.rearrange("p (e w) -> p e w", e=2)[:, :, cs:QT_W],
                        mybir.ActivationFunctionType.Exp,
                        bias=0.0, scale=0.125)
                    if r >= 0:
                        # zero strictly-upper part of the diagonal block
                        # (both heads at once): valid iff f_local >= p_idx
                        nc.gpsimd.affine_select(
                            pt.rearrange("p (e w) -> p e w", e=2)
                              [:, :, cs:cs + 128],
                            pt.rearrange("p (e w) -> p e w", e=2)
                              [:, :, cs:cs + 128],
                            pattern=[[0, 2], [1, 128]],
                            compare_op=mybir.AluOpType.is_ge,
                            fill=0.0,
                            base=0,
                            channel_multiplier=-1)

                    def mk_pv(kb=kb, cs=cs, pt=pt):
                        def f():
                            for e in range(2):
                                h = 2 * p + e
                                nc.tensor.matmul(
                                    ps_pv[e][:, cs:QT_W],
                                    v_ts[kb][:, h * VROW:(h + 1) * VROW],
                                    pt[:, e * QT_W + cs:(e + 1) * QT_W],
                                    start=(kb == 0), stop=(kb == nkb - 1))
                        return f

                    pvq.append(mk_pv())
                    if kb > 0:
                        fill(3)
                        pvq.popleft()()   # PV of k-block kb-1
                    else:
                        fill(2)
                fill(2)
                pvq.popleft()()           # PV of the last k-block
                # normalize: AOT[p][e*64:(e+1)*64, qt] = pv[0:64]/pv[64].
                # stage to SBUF first (frees the PSUM bank), then reciprocal
                # row64 -> row0 of a 1-row tile (partition_broadcast only
                # broadcasts physical partition 0 on HW), broadcast, multiply.
                stg = [None, None]
                rc = [None, None]
                bc = [None, None]
                for e in range(2):
                    stg[e] = stg_pool.tile([VROW, QT_W], F32, tag=f"stg{e}",
                                           name=f"stg{e}")
                    nc.vector.tensor_copy(stg[e][:], ps_pv[e][:])
                    rc[e] = rc_pool.tile([1, QT_W], F32, tag=f"rc{e}",
                                         name=f"rc{e}")
                    nc.vector.reciprocal(rc[e][0:1, :], stg[e][64:65, :])
                    bc[e] = bc_pool.tile([64, QT_W], F32, tag=f"bc{e}",
                                         name=f"bc{e}")
                    nc.gpsimd.partition_broadcast(bc[e][:], rc[e][0:1, :],
                                                  channels=64)
                for e in range(2):
                    nc.vector.tensor_mul(
                        aot_t[p][e * 64:(e + 1) * 64,
                                 qt * QT_W:(qt + 1) * QT_W],
                        stg[e][0:64, :], bc[e][:])

            # ---------------- schedule -----------------------------------
            # Pair 0 needs its own Q/K tiles and the first V chunks before
            # its attention can start; later work is pushed as fillers.
            qk_block(0, 0)
            for sc in range(4):
                v_block(sc)
            for st in range(1, NQT):
                push_qk_units(0, st)

            for p in range(NPAIR):
                for qt in range(NQT):
                    flush()
                    if p == 0 and qt < NQT - 1:
                        for sc in range(4 * qt + 4, 4 * qt + 8):
                            push_v_units(sc)
                    if p + 1 < NPAIR:
                        push_qk_units(p + 1, qt)
                    if p == NPAIR - 1 and qt > 0:
                        push_o_units(qt - 1)
                    attention(p, qt)
            flush()
            push_o_units(NQT - 1)
            flush()

    nc.compile()
    return nc


def make_in_maps(residual, W_Q, W_K, W_V, W_O):
    """Shard + pre-transpose + bf16-cast inputs for the 8 cores."""
    in_maps = []
    for c in range(NCORES):
        b = c // 2
        h0 = (c % 2) * HPC
        sl = slice(h0, h0 + HPC)
        xt = np.ascontiguousarray(residual[b].T).astype(NPBF16)
        wqt = np.ascontiguousarray(
            W_Q[sl].transpose(2, 0, 1).reshape(D, HD)).astype(NPBF16)
        wkt = np.ascontiguousarray(
            W_K[sl].transpose(2, 0, 1).reshape(D, HD)).astype(NPBF16)
        wvt = np.ascontiguousarray(
            W_V[sl].transpose(2, 0, 1).reshape(D, HD)).astype(NPBF16)
        wos = np.ascontiguousarray(W_O[sl].reshape(HD, D)).astype(NPBF16)
        in_maps.append({"xt": xt, "wqt": wqt, "wkt": wkt,
                        "wvt": wvt, "wos": wos})
    return in_maps


def kernel(residual, W_Q, W_K, W_V, W_O, _trace=False):
    residual = np.asarray(residual, dtype=np.float32)
    W_Q = np.asarray(W_Q, dtype=np.float32)
    W_K = np.asarray(W_K, dtype=np.float32)
    W_V = np.asarray(W_V, dtype=np.float32)
    W_O = np.asarray(W_O, dtype=np.float32)

    if "nc" not in _CACHED:
        _CACHED["nc"] = build_kernel()
    nc = _CACHED["nc"]

    in_maps = make_in_maps(residual, W_Q, W_K, W_V, W_O)
    res = run_bass_kernel_spmd(
        nc, in_maps, core_ids=list(range(NCORES)), trace=_trace)
    _CACHED["last_result"] = res

    out = np.empty((B, S, D), dtype=np.float32)
    for b in range(B):
        ot = res.results[2 * b]["ot"] + res.results[2 * b + 1]["ot"]
        out[b] = ot.T
    return out


# revision 17
# speedup vs baseline: 1.0152x; 1.0152x over previous
"""Causal multi-head attention (B=4, S=2048, D=1024, H=16, Dh=64) on 8 TRN2
NeuronCores.

Sharding: core c -> batch b = c//2, head group hg = c%2 (8 heads each).
Each core computes the partial output (sum over its 8 heads) TRANSPOSED:
OT_partial [D=1024, S=2048] in fp32.  Host sums the two partials per batch
and transposes back.

v2: software-pipelined issue order.  The attention inner loop previously
issued scores -> exp -> mask -> PV per k-block in program order, so the
in-order PE queue stalled behind ACT's exp every k-block.  Now PV lags one
k-block behind scores, and projection matmuls (Q/K/V/O) are issued as
single-MM "filler" units between the scores and PV of each k-block, so PE
always has independent work while ACT computes exp.  Fillers pending at a
(pair, q-tile) boundary are flushed there, which keeps the producer-before-
consumer order that the in-order engine queues need to stay deadlock-free.

Per-core kernel (bf16 matmuls, fp32 PSUM accumulation):
  XT  = residual[b].T (bf16)          [1024(m), 2048(s)]  (host-pretransposed)
  WQT/WKT/WVT (bf16) [1024(m), 512(h*64+d)]              (host-pretransposed)
  WOS = W_O[heads].reshape (bf16)     [512(h*64+d), 1024(m)]
  QT/KT (pair-packed) [128=2x64(hd), 2048(s)] x 4 pairs; score matmuls are
      row-tiled (heads at array rows 0-63 / 64-127)
  V    [128(k within chunk), 8*65] x 16 chunks (ones col per head: the PV
      matmul's 65th output row accumulates sum(exp) for free)
  scoresT [k,q] tiles -> one ACT exp per k-block covering both heads
      (scale=1/8; no max-subtraction -- scores are bounded ~+-2.5 for this
      input distribution)
  causal: staircase-restricted matmul widths + affine_select on the
      diagonal 128-blocks only
  PV: psum[65, 512] accumulated over k blocks; row 64 = sumexp
  normalize (partition_broadcast only works from physical partition 0 on
      HW): stage psum->SBUF, DVE reciprocal row64->row0 of a 1-row tile,
      gpsimd partition_broadcast from p0, DVE multiply
  O-proj: lhsT = WOS chunks, rhs = AOT pair tiles -> OT [1024, 2048] fp32
"""

from collections import deque
from contextlib import ExitStack

import ml_dtypes
import numpy as np

import concourse.bacc as bacc
import concourse.mybir as mybir
import concourse.tile as tile
from concourse.bass_utils import run_bass_kernel_spmd

# ---------------------------------------------------------------- constants
B, S, D = 4, 2048, 1024
H, Dh = 16, 64
NCORES = 8
HPC = H // 2          # heads per core = 8
HD = HPC * Dh         # 512
NPAIR = HPC // 2      # 4 head pairs per core
MC = D // 128         # 8 m-chunks
QT_W = 512            # q tile width
NQT = S // QT_W       # 4
SC = S // 128         # 16 s-chunks (k blocks)
VROW = Dh + 1         # 65: per-head V columns incl. ones col
F32 = mybir.dt.float32
BF16 = mybir.dt.bfloat16
NPBF16 = ml_dtypes.bfloat16

_CACHED = {}


def build_kernel(repeat=1, hw_loop=False):
    nc = bacc.Bacc("TRN2", target_bir_lowering=False, debug=False,
                   num_devices=NCORES)

    xt_d = nc.dram_tensor("xt", [D, S], BF16, kind="ExternalInput").ap()
    wqt_d = nc.dram_tensor("wqt", [D, HD], BF16, kind="ExternalInput").ap()
    wkt_d = nc.dram_tensor("wkt", [D, HD], BF16, kind="ExternalInput").ap()
    wvt_d = nc.dram_tensor("wvt", [D, HD], BF16, kind="ExternalInput").ap()
    wos_d = nc.dram_tensor("wos", [HD, D], BF16, kind="ExternalInput").ap()
    ot_d = nc.dram_tensor("ot", [D, S], F32, kind="ExternalOutput").ap()

    with tile.TileContext(nc) as tc, ExitStack() as ctx:
        if hw_loop and repeat > 1:
            ctx.enter_context(tc.For_i(0, repeat, 1, name="rep"))
            repeat = 1
        # ---------------- persistent SBUF tensors -------------------------
        w_pool = ctx.enter_context(tc.tile_pool(name="w", bufs=1))
        qk_pool = ctx.enter_context(tc.tile_pool(name="qk", bufs=1))
        v_pool = ctx.enter_context(tc.tile_pool(name="v", bufs=1))
        aot_pool = ctx.enter_context(tc.tile_pool(name="aot", bufs=1))
        xt_pool = ctx.enter_context(tc.tile_pool(name="xt", bufs=1))
        pt_pool = ctx.enter_context(tc.tile_pool(name="pt", bufs=6))
        stg_pool = ctx.enter_context(tc.tile_pool(name="stg", bufs=6))
        rc_pool = ctx.enter_context(tc.tile_pool(name="rc", bufs=4))
        bc_pool = ctx.enter_context(tc.tile_pool(name="bc", bufs=4))
        psum = ctx.enter_context(tc.tile_pool(name="ps", bufs=1,
                                              space="PSUM"))

        for _rep in range(repeat):
            wq_t = w_pool.tile([128, MC * HD], BF16, tag="wqt")
            wk_t = w_pool.tile([128, MC * HD], BF16, tag="wkt")
            wv_t = w_pool.tile([128, MC * HD], BF16, tag="wvt")
            wo_t = w_pool.tile([128, NPAIR * D], BF16, tag="wot")
            xt_t = xt_pool.tile([128, MC * S], BF16)
            # wq + xt first: the opening qk_block(0,0) only needs those, so
            # front-loading them shortens the startup DMA wait
            for mc in range(MC):
                nc.sync.dma_start(wq_t[:, mc * HD:(mc + 1) * HD],
                                  wqt_d[mc * 128:(mc + 1) * 128, :])
                nc.sync.dma_start(xt_t[:, mc * S:(mc + 1) * S],
                                  xt_d[mc * 128:(mc + 1) * 128, :])
            for mc in range(MC):
                nc.sync.dma_start(wk_t[:, mc * HD:(mc + 1) * HD],
                                  wkt_d[mc * 128:(mc + 1) * 128, :])
                nc.sync.dma_start(wv_t[:, mc * HD:(mc + 1) * HD],
                                  wvt_d[mc * 128:(mc + 1) * 128, :])
            for c in range(NPAIR):
                nc.sync.dma_start(wo_t[:, c * D:(c + 1) * D],
                                  wos_d[c * 128:(c + 1) * 128, :])

            # QT/KT pair-packed: [128 (2 heads x 64), S] per pair
            qt_t = [qk_pool.tile([128, S], BF16, tag=f"qt{p}", name=f"qt{p}")
                    for p in range(NPAIR)]
            kt_t = [qk_pool.tile([128, S], BF16, tag=f"kt{p}", name=f"kt{p}")
                    for p in range(NPAIR)]
            # V: per s-chunk [128, HPC*VROW]: 8 heads x (64 data + 1 ones)
            v_ts = [v_pool.tile([128, HPC * VROW], BF16, tag=f"v{sc}",
                                name=f"v{sc}") for sc in range(SC)]
            # AOT pair-packed: [128, S] per pair
            aot_t = [aot_pool.tile([128, S], BF16, tag=f"aot{p}",
                                   name=f"aot{p}") for p in range(NPAIR)]

            # -------- filler machinery: single-MM units issued into the
            # gaps of the attention stream ---------------------------------
            fillers = deque()

            def fill(n):
                for _ in range(min(n, len(fillers))):
                    fillers.popleft()()

            def flush():
                while fillers:
                    fillers.popleft()()

            # ---------------- Q/K projection units for one (pair, s-tile)
            def push_qk_units(p, st):
                cell = {}

                def mk_mm(which, mc):
                    def f():
                        if mc == 0:
                            cell[which] = psum.tile(
                                [128, QT_W], F32, tag="qk", bufs=2,
                                name=f"ps_{which}")
                        w = wq_t if which == "q" else wk_t
                        nc.tensor.matmul(
                            cell[which][:],
                            w[:, mc * HD + p * 128: mc * HD + (p + 1) * 128],
                            xt_t[:, mc * S + st * QT_W:
                                 mc * S + (st + 1) * QT_W],
                            start=(mc == 0), stop=(mc == MC - 1))
                    return f

                def mk_copy(which):
                    def f():
                        dst = qt_t[p] if which == "q" else kt_t[p]
                        nc.vector.tensor_copy(
                            dst[:, st * QT_W:(st + 1) * QT_W], cell[which][:])
                    return f

                for mc in range(MC):
                    fillers.append(mk_mm("q", mc))
                for mc in range(MC):
                    fillers.append(mk_mm("k", mc))
                fillers.append(mk_copy("q"))
                fillers.append(mk_copy("k"))

            def qk_block(p, st):
                push_qk_units(p, st)
                flush()

            # ---------------- V projection units (activations stationary)
            def push_v_units(sc):
                cell = {}

                def mk_mm(mc):
                    def f():
                        if mc == 0:
                            cell["v"] = psum.tile(
                                [128, HD], F32, tag="qk", bufs=2, name="ps_v")
                        nc.tensor.matmul(
                            cell["v"][:],
                            xt_t[:, mc * S + sc * 128: mc * S + (sc + 1) * 128],
                            wv_t[:, mc * HD:(mc + 1) * HD],
                            start=(mc == 0), stop=(mc == MC - 1))
                    return f

                def cp():
                    vg = v_ts[sc][:].rearrange("p (h e) -> p h e", h=HPC)
                    nc.vector.tensor_copy(
                        vg[:, :, 0:Dh],
                        cell["v"][:].rearrange("p (h d) -> p h d", h=HPC))
                    nc.vector.memset(vg[:, :, Dh:VROW], 1.0)

                for mc in range(MC):
                    fillers.append(mk_mm(mc))
                fillers.append(cp)

            def v_block(sc):
                push_v_units(sc)
                flush()

            # ---------------- O-projection units (one q-tile, all m) ------
            def push_o_units(ot):
                for mc in range(MC):
                    cell = {}

                    def mk_mm(c, mc=mc, cell=cell):
                        def f():
                            if c == 0:
                                cell["o"] = psum.tile(
                                    [128, QT_W], F32, tag="qk", bufs=2,
                                    name="ps_o")
                            nc.tensor.matmul(
                                cell["o"][:],
                                wo_t[:, c * D + mc * 128:
                                     c * D + (mc + 1) * 128],
                                aot_t[c][:, ot * QT_W:(ot + 1) * QT_W],
                                start=(c == 0), stop=(c == NPAIR - 1))
                        return f

                    def cp(mc=mc, cell=cell):
                        ot_sb = pt_pool.tile([128, QT_W], F32, tag="ott",
                                             bufs=4, name="ot_sb")
                        nc.vector.tensor_copy(ot_sb[:], cell["o"][:])
                        nc.sync.dma_start(
                            ot_d[mc * 128:(mc + 1) * 128,
                                 ot * QT_W:(ot + 1) * QT_W], ot_sb[:])

                    for c in range(NPAIR):
                        fillers.append(mk_mm(c))
                    fillers.append(cp)

            # ---------------- attention for (pair, q-tile) ---------------
            def attention(p, qt):
                nkb = 4 * qt + 4
                ps_pv = [psum.tile([VROW, QT_W], F32, tag=f"pv{e}", bufs=1,
                                   name=f"ps_pv{e}") for e in range(2)]
                pvq = deque()
                for kb in range(nkb):
                    r = kb - 4 * qt
                    cs = max(0, r * 128)  # first valid q col in tile
                    # both heads' scoresT into one 2-bank psum tile
                    ps_s = psum.tile([128, 2 * QT_W], F32, tag="s", bufs=2,
                                     name="ps_s")
                    pt = pt_pool.tile([128, 2 * QT_W], BF16, tag="pt",
                                      name="pt")
                    for e in range(2):
                        hb = e * 64
                        nc.tensor.matmul(
                            ps_s[:, e * QT_W + cs:(e + 1) * QT_W],
                            kt_t[p][hb:hb + 64, kb * 128:(kb + 1) * 128],
                            qt_t[p][hb:hb + 64,
                                    qt * QT_W + cs:(qt + 1) * QT_W],
                            start=True, stop=True)
                    # one exp(scores/8) PSUM -> SBUF for both heads
                    nc.scalar.activation(
                        pt.rearrange("p (e w) -> p e w", e=2)[:, :, cs:QT_W],
                        ps_s.rearrange("p (e w) -> p e w", e=2)[:, :, cs:QT_W],
                        mybir.ActivationFunctionType.Exp,
                        bias=0.0, scale=0.125)
                    if r >= 0:
                        # zero strictly-upper part of the diagonal block
                        # (both heads at once): valid iff f_local >= p_idx
                        nc.gpsimd.affine_select(
                            pt.rearrange("p (e w) -> p e w", e=2)
                              [:, :, cs:cs + 128],
                            pt.rearrange("p (e w) -> p e w", e=2)
                              [:, :, cs:cs + 128],
                            pattern=[[0, 2], [1, 128]],
                            compare_op=mybir.AluOpType.is_ge,
                            fill=0.0,
                            base=0,
                            channel_multiplier=-1)

                    def mk_pv(kb=kb, cs=cs, pt=pt):
                        def f():
                            for e in range(2):
                                h = 2 * p + e
                                nc.tensor.matmul(
                                    ps_pv[e][:, cs:QT_W],
                                    v_ts[kb][:, h * VROW:(h + 1) * VROW],
                                    pt[:, e * QT_W + cs:(e + 1) * QT_W],
                                    start=(kb == 0), stop=(kb == nkb - 1))
                        return f

                    pvq.append(mk_pv())
                    if kb > 0:
                        fill(3)
                        pvq.popleft()()   # PV of k-block kb-1
                    else:
                        fill(2)
                fill(2)
                pvq.popleft()()           # PV of the last k-block
                # normalize: AOT[p][e*64:(e+1)*64, qt] = pv[0:64]/pv[64].
                # stage to SBUF first (frees the PSUM bank), then reciprocal
                # row64 -> row0 of a 1-row tile (partition_broadcast only
                # broadcasts physical partition 0 on HW), broadcast, multiply.
                stg = [None, None]
                rc = [None, None]
                bc = [None, None]
                for e in range(2):
                    stg[e] = stg_pool.tile([VROW, QT_W], F32, tag=f"stg{e}",
                                           name=f"stg{e}")
                    nc.vector.tensor_copy(stg[e][:], ps_pv[e][:])
                    rc[e] = rc_pool.tile([1, QT_W], F32, tag=f"rc{e}",
                                         name=f"rc{e}")
                    nc.vector.reciprocal(rc[e][0:1, :], stg[e][64:65, :])
                    bc[e] = bc_pool.tile([64, QT_W], F32, tag=f"bc{e}",
                                         name=f"bc{e}")
                    nc.gpsimd.partition_broadcast(bc[e][:], rc[e][0:1, :],
                                                  channels=64)
                for e in range(2):
                    nc.vector.tensor_mul(
                        aot_t[p][e * 64:(e + 1) * 64,
                                 qt * QT_W:(qt + 1) * QT_W],
                        stg[e][0:64, :], bc[e][:])

            # ---------------- schedule -----------------------------------
            # Pair 0 needs its own Q/K tiles and the first V chunks before
            # its attention can start; later work is pushed as fillers.
            qk_block(0, 0)
            for sc in range(4):
                v_block(sc)
            for st in range(1, NQT):
                push_qk_units(0, st)

            for p in range(NPAIR):
                for qt in range(NQT):
                    flush()
                    if p == 0 and qt < NQT - 1:
                        for sc in range(4 * qt + 4, 4 * qt + 8):
                            push_v_units(sc)
                    if p + 1 < NPAIR:
                        push_qk_units(p + 1, qt)
                    if p == NPAIR - 1 and qt > 0:
                        push_o_units(qt - 1)
                    attention(p, qt)
            flush()
            push_o_units(NQT - 1)
            flush()

    nc.compile()
    return nc


def make_in_maps(residual, W_Q, W_K, W_V, W_O):
    """Shard + pre-transpose + bf16-cast inputs for the 8 cores."""
    in_maps = []
    for c in range(NCORES):
        b = c // 2
        h0 = (c % 2) * HPC
        sl = slice(h0, h0 + HPC)
        xt = np.ascontiguousarray(residual[b].T).astype(NPBF16)
        wqt = np.ascontiguousarray(
            W_Q[sl].transpose(2, 0, 1).reshape(D, HD)).astype(NPBF16)
        wkt = np.ascontiguousarray(
            W_K[sl].transpose(2, 0, 1).reshape(D, HD)).astype(NPBF16)
        wvt = np.ascontiguousarray(
            W_V[sl].transpose(2, 0, 1).reshape(D, HD)).astype(NPBF16)
        wos = np.ascontiguousarray(W_O[sl].reshape(HD, D)).astype(NPBF16)
        in_maps.append({"xt": xt, "wqt": wqt, "wkt": wkt,
                        "wvt": wvt, "wos": wos})
    return in_maps


def kernel(residual, W_Q, W_K, W_V, W_O, _trace=False):
    residual = np.asarray(residual, dtype=np.float32)
    W_Q = np.asarray(W_Q, dtype=np.float32)
    W_K = np.asarray(W_K, dtype=np.float32)
    W_V = np.asarray(W_V, dtype=np.float32)
    W_O = np.asarray(W_O, dtype=np.float32)

    if "nc" not in _CACHED:
        _CACHED["nc"] = build_kernel()
    nc = _CACHED["nc"]

    in_maps = make_in_maps(residual, W_Q, W_K, W_V, W_O)
    res = run_bass_kernel_spmd(
        nc, in_maps, core_ids=list(range(NCORES)), trace=_trace)
    _CACHED["last_result"] = res

    out = np.empty((B, S, D), dtype=np.float32)
    for b in range(B):
        ot = res.results[2 * b]["ot"] + res.results[2 * b + 1]["ot"]
        out[b] = ot.T
    return out
